# revision 10
# baseline (speedup 1.0000x reference)
"""CTLSTM cell fused kernel for 8 Trainium2 NeuronCores — v3.

Strategy (data-parallel over batch, transposed compute layout):
  - B=16384 rows sharded 2048/core; weights replicated.
  - TRANSPOSED GEMM: weights are the PE stationary operand, batch streams
    as moving data, so the output lands as [gate_partition, batch_free].
  - Gates on partitions means the per-gate bias is a PER-PARTITION vector:
    ACT fuses bias-add + nonlinearity in ONE pass directly from PSUM
    (out = act(psum + bias[p])), no DVE bias pass.
  - All gate tiles / ct / outputs are bf16: DVE tensor ops hit the 2x_1p
    mode (2 elem/cycle/lane), and output DMA traffic halves.
  - Gate order i, f, z, o, ib, d, fb: z drains early so the c / tanh(c) /
    h epilogue overlaps the remaining gate sweeps on DVE/ACT while the PE
    keeps streaming; fb last leaves only the short cb chain as tail.
  - decay_rate = softplus(wd) has no ACT table: sigmoid(-wd) during the
    main sigmoid run (scale=-1, host-negated bias), then -ln(.) with the
    Ln pass chained before the fb drains.
  - Batch processed in 2 slabs of 1024/core; PSUM: [128,1024] accumulators
    (2 banks), pool of 4, gate-blocks swept k-outer in groups of 3 so the
    first slab pipelines against the streaming weight DMAs.
"""

import numpy as np
import ml_dtypes

import concourse.bacc as bacc
import concourse.bass as bass
import concourse.mybir as mybir
import concourse.tile as tile
from concourse.tile_rust import add_dep_helper
from concourse.bass_utils import run_bass_kernel_spmd

NCORES = 8
B = 16384
I = 512
H = 512
NG = 7
G = NG * H          # 3584
K2 = I + H          # 1024
P = 128
BS = B // NCORES    # 2048 batch cols per core
SLAB = 1024         # batch cols per slab
NSLAB = BS // SLAB  # 2
NGB = G // P        # 28 gate-blocks of 128
NHB = H // P        # 4 h-blocks
NK = K2 // P        # 8 contraction chunks

BF16 = mybir.dt.bfloat16
F32 = mybir.dt.float32
AF = mybir.ActivationFunctionType
NPBF16 = ml_dtypes.bfloat16

# new gate order -> reference gate index (reference: i,f,z,o,d,ib,fb)
PERM = [0, 1, 2, 3, 5, 4, 6]
GI_I, GI_F, GI_Z, GI_O, GI_IB, GI_D, GI_FB = range(7)
GATE_FUNC = [AF.Sigmoid, AF.Sigmoid, AF.Tanh, AF.Sigmoid, AF.Sigmoid,
             AF.Sigmoid, AF.Sigmoid]

# gate-block sweep groups (PSUM: 3 live accumulators x 2 banks + slack)
GB_GROUPS = [list(range(s, min(s + 3, NGB))) for s in range(0, NGB, 3)]

TRACE = False
LAST_RESULTS = None

_nc_cache = None


def _build():
    nc = bacc.Bacc("TRN2", target_bir_lowering=False, debug=False)

    xh = nc.dram_tensor("xh", [K2, BS], BF16, kind="ExternalInput")
    w2 = nc.dram_tensor("w2", [K2, G], BF16, kind="ExternalInput")
    ctT = nc.dram_tensor("ctT", [H, BS], BF16, kind="ExternalInput")
    bias_d = nc.dram_tensor("bias", [P, NGB], F32, kind="ExternalInput")

    h_d = nc.dram_tensor("h", [H, BS], BF16, kind="ExternalOutput")
    c_d = nc.dram_tensor("c", [H, BS], BF16, kind="ExternalOutput")
    cb_d = nc.dram_tensor("cb", [H, BS], BF16, kind="ExternalOutput")
    o_d = nc.dram_tensor("o", [H, BS], BF16, kind="ExternalOutput")
    dr_d = nc.dram_tensor("dr", [H, BS], BF16, kind="ExternalOutput")

    with tile.TileContext(nc) as tc:
        with (
            tc.tile_pool(name="wp", bufs=1) as wp,
            tc.tile_pool(name="cp", bufs=1) as cp,
            tc.tile_pool(name="xp", bufs=2) as xp,
            tc.tile_pool(name="ctp", bufs=2) as ctp,
            tc.tile_pool(name="gp", bufs=1) as gp,
            tc.tile_pool(name="pp", bufs=4, space=bass.MemorySpace.PSUM) as pp,
        ):
            # weight chunks [128, 3584] bf16, resident. The critical first
            # wave (first w quarter + slab-0 xh + bias) streams alone; all
            # later DMA is dep-chained behind it so it cannot steal HBM
            # bandwidth from the data the first gb sweeps are waiting on.
            WSPLIT = 1280
            w_sb = [wp.tile([P, G], BF16, tag=f"w{k}", name=f"w{k}")
                    for k in range(NK)]
            bb = cp.tile([P, NGB], F32, tag="bb")

            prev_act = None  # ACT program-order chain (table grouping)

            def chain(a):
                nonlocal prev_act
                if prev_act is not None:
                    add_dep_helper(a.ins, prev_act.ins, reason="act order")
                prev_act = a

            wave1 = []
            xh_slabs = []
            for s in range(NSLAB):
                t = [xp.tile([P, SLAB], BF16, tag=f"xh{k}", name=f"xh{k}")
                     for k in range(NK)]
                xh_slabs.append(t)
            for k in range(NK):
                wave1.append(nc.sync.dma_start(
                    w_sb[k][:, 0:WSPLIT], w2[k * P:(k + 1) * P, 0:WSPLIT]))
                for h2 in range(2):
                    csl = slice(h2 * 512, (h2 + 1) * 512)
                    wave1.append(nc.sync.dma_start(
                        xh_slabs[0][k][:, csl], xh[k * P:(k + 1) * P, csl]))
            wave1.append(nc.sync.dma_start(bb[:], bias_d[:]))

            def after_wave1(dma):
                for w1 in wave1:
                    add_dep_helper(dma.ins, w1.ins, reason="dma priority")
                return dma

            for k in range(NK):
                after_wave1(nc.sync.dma_start(
                    w_sb[k][:, WSPLIT:G], w2[k * P:(k + 1) * P, WSPLIT:G]))

            for s in range(NSLAB):
                ssl = slice(s * SLAB, (s + 1) * SLAB)
                xh_s = xh_slabs[s]
                if s > 0:
                    for k in range(NK):
                        after_wave1(nc.sync.dma_start(
                            xh_s[k][:], xh[k * P:(k + 1) * P, ssl]))

                cts = []
                for hb in range(NHB):
                    t_ = ctp.tile([P, SLAB], BF16, tag=f"ct{hb}")
                    after_wave1(nc.sync.dma_start(
                        t_[:], ctT[hb * P:(hb + 1) * P, ssl]))
                    cts.append(t_)

                # gate tiles for this slab (bf16, reused in-place later)
                ga = [gp.tile([P, SLAB], BF16, tag=f"ga{gb}", name=f"ga{gb}")
                      for gb in range(NGB)]

                def T(g, hb):
                    return ga[g * NHB + hb]

                def out_dma(dst, hb, src):
                    nc.sync.dma_start(dst[hb * P:(hb + 1) * P, ssl], src[:])

                # per-hb epilogue emitters, run as soon as a gate-block
                # drains so only hb3's short cb chain trails the last MM
                def epi_z(hb):
                    # c = f*ct + i*z into the f tile
                    F, Ii, Z = T(GI_F, hb), T(GI_I, hb), T(GI_Z, hb)
                    nc.vector.tensor_mul(F[:], F[:], cts[hb][:])
                    nc.vector.tensor_mul(Ii[:], Ii[:], Z[:])
                    nc.vector.tensor_add(F[:], F[:], Ii[:])
                    out_dma(c_d, hb, F)
                    if hb == NHB - 1:
                        # tanh(c) into the i tiles; batched to keep the ACT
                        # chain from bubbling on the first DVE c
                        for hb2 in range(NHB):
                            chain(nc.scalar.activation(T(GI_I, hb2)[:],
                                                       T(GI_F, hb2)[:],
                                                       AF.Tanh))

                def epi_o(hb):
                    # h = o * tanh(c) into the tanh(c) (= i) tile
                    Tc = T(GI_I, hb)
                    nc.vector.tensor_mul(Tc[:], Tc[:], T(GI_O, hb)[:])
                    out_dma(h_d, hb, Tc)

                def epi_d(hb):
                    # decay_rate = -ln(sigmoid(-wd)); one Ln table window
                    if hb == NHB - 1:
                        for hb2 in range(NHB):
                            S = T(GI_D, hb2)
                            chain(nc.scalar.activation(S[:], S[:], AF.Ln))
                            nc.vector.tensor_scalar_mul(S[:], S[:], -1.0)
                            out_dma(dr_d, hb2, S)

                def epi_fb(hb):
                    # cbar = fb*ct + ib*z into the fb tile
                    FB, IB, Z = T(GI_FB, hb), T(GI_IB, hb), T(GI_Z, hb)
                    nc.vector.tensor_mul(IB[:], IB[:], Z[:])
                    nc.vector.tensor_mul(FB[:], FB[:], cts[hb][:])
                    nc.vector.tensor_add(FB[:], FB[:], IB[:])
                    out_dma(cb_d, hb, FB)

                epilogue = {GI_Z: epi_z, GI_O: epi_o,
                            GI_D: epi_d, GI_FB: epi_fb}

                # ---- GEMM + fused bias/activation drain ----
                for grp in GB_GROUPS:
                    accs = {gb: pp.tile([P, SLAB], F32, tag="acc", name="acc")
                            for gb in grp}
                    for k in range(NK):
                        for gb in grp:
                            stat = w_sb[k][:, gb * P:(gb + 1) * P]
                            for h2 in range(SLAB // 512):
                                csl = slice(h2 * 512, (h2 + 1) * 512)
                                nc.tensor.matmul(
                                    accs[gb][:, csl], stat, xh_s[k][:, csl],
                                    start=(k == 0), stop=(k == NK - 1),
                                )
                    for gb in grp:
                        g, hb = gb // NHB, gb % NHB
                        chain(nc.scalar.activation(
                            ga[gb][:], accs[gb][:], GATE_FUNC[g],
                            scale=-1.0 if g == GI_D else 1.0,
                            bias=bb[:, gb:gb + 1]))
                        if g == GI_O:
                            out_dma(o_d, hb, ga[gb])
                        fn = epilogue.get(g)
                        if fn is not None:
                            fn(hb)

    nc.compile()
    return nc


def kernel(x, ht, ct, Wx, bx, Wh, bh):
    global _nc_cache, LAST_RESULTS
    if _nc_cache is None:
        _nc_cache = _build()
    nc = _nc_cache

    x = np.ascontiguousarray(x, dtype=np.float32)
    ht = np.ascontiguousarray(ht, dtype=np.float32)
    ct = np.ascontiguousarray(ct, dtype=np.float32)

    # host staging: transpose/concat/cast + gate permutation
    xh_full = np.empty((K2, B), dtype=NPBF16)
    xh_full[:I, :] = x.T.astype(NPBF16)
    xh_full[I:, :] = ht.T.astype(NPBF16)
    ctT_full = np.ascontiguousarray(ct.T.astype(NPBF16))

    WxT = np.asarray(Wx, dtype=np.float32).T   # [512, 3584]
    WhT = np.asarray(Wh, dtype=np.float32).T
    bsum = np.asarray(bx, dtype=np.float32) + np.asarray(bh, dtype=np.float32)
    w2 = np.empty((K2, G), dtype=NPBF16)
    bbp = np.empty(G, dtype=np.float32)
    for n, old in enumerate(PERM):
        dsl = slice(n * H, (n + 1) * H)
        ssl = slice(old * H, (old + 1) * H)
        w2[:I, dsl] = WxT[:, ssl].astype(NPBF16)
        w2[I:, dsl] = WhT[:, ssl].astype(NPBF16)
        # d-gate ACT runs with scale=-1: out = sigmoid(-wd) needs -bias
        bbp[dsl] = -bsum[ssl] if n == GI_D else bsum[ssl]
    bias = np.ascontiguousarray(bbp.reshape(NGB, P).T)  # [128, 28]

    in_maps = []
    for cidx in range(NCORES):
        sl = slice(cidx * BS, (cidx + 1) * BS)
        in_maps.append({
            "xh": np.ascontiguousarray(xh_full[:, sl]),
            "w2": w2,
            "ctT": np.ascontiguousarray(ctT_full[:, sl]),
            "bias": bias,
        })

    res = run_bass_kernel_spmd(nc, in_maps, core_ids=list(range(NCORES)),
                               trace=TRACE)
    LAST_RESULTS = res

    outs = {}
    for name in ("h", "c", "cb", "o", "dr"):
        outs[name] = np.concatenate(
            [np.asarray(res.results[cidx][name]).T.astype(np.float32)
             for cidx in range(NCORES)], axis=0
        )
    return outs["h"], outs["c"], outs["cb"], outs["o"], outs["dr"]


# revision 14
# speedup vs baseline: 1.0217x; 1.0217x over previous
"""CTLSTM cell fused kernel for 8 Trainium2 NeuronCores — v3.

Strategy (data-parallel over batch, transposed compute layout):
  - B=16384 rows sharded 2048/core; weights replicated.
  - TRANSPOSED GEMM: weights are the PE stationary operand, batch streams
    as moving data, so the output lands as [gate_partition, batch_free].
  - Gates on partitions means the per-gate bias is a PER-PARTITION vector:
    ACT fuses bias-add + nonlinearity in ONE pass directly from PSUM
    (out = act(psum + bias[p])), no DVE bias pass.
  - All gate tiles / ct / outputs are bf16: DVE tensor ops hit the 2x_1p
    mode (2 elem/cycle/lane), and output DMA traffic halves.
  - Gate order i, f, z, o, ib, d, fb: z drains early so the c / tanh(c) /
    h epilogue overlaps the remaining gate sweeps on DVE/ACT while the PE
    keeps streaming; fb last leaves only the short cb chain as tail.
  - decay_rate = softplus(wd) has no ACT table: sigmoid(-wd) during the
    main sigmoid run (scale=-1, host-negated bias), then -ln(.) with the
    Ln pass chained before the fb drains.
  - Batch processed in 2 slabs of 1024/core; PSUM: [128,1024] accumulators
    (2 banks), pool of 4, gate-blocks swept k-outer in groups of 3 so the
    first slab pipelines against the streaming weight DMAs.
"""

import numpy as np
import ml_dtypes

import concourse.bacc as bacc
import concourse.bass as bass
import concourse.mybir as mybir
import concourse.tile as tile
from concourse.tile_rust import add_dep_helper
from concourse.bass_utils import run_bass_kernel_spmd

NCORES = 8
B = 16384
I = 512
H = 512
NG = 7
G = NG * H          # 3584
K2 = I + H          # 1024
P = 128
BS = B // NCORES    # 2048 batch cols per core
SLAB = 1024         # batch cols per slab
NSLAB = BS // SLAB  # 2
NGB = G // P        # 28 gate-blocks of 128
NHB = H // P        # 4 h-blocks
NK = K2 // P        # 8 contraction chunks

BF16 = mybir.dt.bfloat16
F32 = mybir.dt.float32
AF = mybir.ActivationFunctionType
NPBF16 = ml_dtypes.bfloat16

# new gate order -> reference gate index (reference: i,f,z,o,d,ib,fb)
PERM = [0, 1, 2, 3, 5, 4, 6]
GI_I, GI_F, GI_Z, GI_O, GI_IB, GI_D, GI_FB = range(7)
GATE_FUNC = [AF.Sigmoid, AF.Sigmoid, AF.Tanh, AF.Sigmoid, AF.Sigmoid,
             AF.Sigmoid, AF.Sigmoid]

# gate-block sweep groups of 2: 2 live accumulators x 2 banks in a 4-slot
# pool, so the next group's accumulators allocate while the previous two
# drain -> no PE stall at group boundaries
GB_GROUPS = [list(range(s, min(s + 2, NGB))) for s in range(0, NGB, 2)]

TRACE = False
LAST_RESULTS = None

_nc_cache = None


def _build():
    nc = bacc.Bacc("TRN2", target_bir_lowering=False, debug=False)

    xh = nc.dram_tensor("xh", [K2, BS], BF16, kind="ExternalInput")
    w2 = nc.dram_tensor("w2", [K2, G], BF16, kind="ExternalInput")
    ctT = nc.dram_tensor("ctT", [H, BS], BF16, kind="ExternalInput")
    bias_d = nc.dram_tensor("bias", [P, NGB], F32, kind="ExternalInput")

    h_d = nc.dram_tensor("h", [H, BS], BF16, kind="ExternalOutput")
    c_d = nc.dram_tensor("c", [H, BS], BF16, kind="ExternalOutput")
    cb_d = nc.dram_tensor("cb", [H, BS], BF16, kind="ExternalOutput")
    o_d = nc.dram_tensor("o", [H, BS], BF16, kind="ExternalOutput")
    dr_d = nc.dram_tensor("dr", [H, BS], BF16, kind="ExternalOutput")

    with tile.TileContext(nc) as tc:
        with (
            tc.tile_pool(name="wp", bufs=1) as wp,
            tc.tile_pool(name="cp", bufs=1) as cp,
            tc.tile_pool(name="xp", bufs=2) as xp,
            tc.tile_pool(name="ctp", bufs=2) as ctp,
            tc.tile_pool(name="gp", bufs=1) as gp,
            tc.tile_pool(name="pp", bufs=4, space=bass.MemorySpace.PSUM) as pp,
        ):
            # weight chunks [128, 3584] bf16, resident. DMA is staged in
            # chained waves so later transfers cannot steal HBM bandwidth
            # from the data the first gb sweeps are waiting on:
            #   wave1: w cols 0:768 + slab-0 xh + bias  (first ~4 groups)
            #   wave2: w cols 768:2048                  (through gb 15)
            #   wave3: w cols 2048:3584 + ct + slab-1 xh
            WS1, WS2 = 768, 2048
            w_sb = [wp.tile([P, G], BF16, tag=f"w{k}", name=f"w{k}")
                    for k in range(NK)]
            bb = cp.tile([P, NGB], F32, tag="bb")

            prev_act = None  # ACT program-order chain (table grouping)

            def chain(a):
                nonlocal prev_act
                if prev_act is not None:
                    add_dep_helper(a.ins, prev_act.ins, reason="act order")
                prev_act = a

            xh_slabs = []
            for s in range(NSLAB):
                t = [xp.tile([P, SLAB], BF16, tag=f"xh{k}", name=f"xh{k}")
                     for k in range(NK)]
                xh_slabs.append(t)

            wave1 = []
            for k in range(NK):
                wave1.append(nc.sync.dma_start(
                    w_sb[k][:, 0:WS1], w2[k * P:(k + 1) * P, 0:WS1]))
                wave1.append(nc.sync.dma_start(
                    xh_slabs[0][k][:], xh[k * P:(k + 1) * P, 0:SLAB]))
            wave1.append(nc.sync.dma_start(bb[:], bias_d[:]))

            def after(prev, dma):
                for p_ in prev:
                    add_dep_helper(dma.ins, p_.ins, reason="dma priority")
                return dma

            wave2 = [after(wave1, nc.sync.dma_start(
                w_sb[k][:, WS1:WS2], w2[k * P:(k + 1) * P, WS1:WS2]))
                for k in range(NK)]
            for k in range(NK):
                after(wave2, nc.sync.dma_start(
                    w_sb[k][:, WS2:G], w2[k * P:(k + 1) * P, WS2:G]))

            ct_slabs = []
            for s in range(NSLAB):
                ssl = slice(s * SLAB, (s + 1) * SLAB)
                if s > 0:
                    for k in range(NK):
                        after(wave2, nc.sync.dma_start(
                            xh_slabs[s][k][:], xh[k * P:(k + 1) * P, ssl]))
                cts_ = []
                for hb in range(NHB):
                    t_ = ctp.tile([P, SLAB], BF16, tag=f"ct{hb}",
                                  name=f"ct{hb}")
                    after(wave2, nc.sync.dma_start(
                        t_[:], ctT[hb * P:(hb + 1) * P, ssl]))
                    cts_.append(t_)
                ct_slabs.append(cts_)

            for s in range(NSLAB):
                ssl = slice(s * SLAB, (s + 1) * SLAB)
                xh_s = xh_slabs[s]
                cts = ct_slabs[s]

                # gate tiles for this slab (bf16, reused in-place later)
                ga = [gp.tile([P, SLAB], BF16, tag=f"ga{gb}", name=f"ga{gb}")
                      for gb in range(NGB)]

                def T(g, hb):
                    return ga[g * NHB + hb]

                def out_dma(dst, hb, src):
                    nc.sync.dma_start(dst[hb * P:(hb + 1) * P, ssl], src[:])

                # per-hb epilogue emitters, run as soon as a gate-block
                # drains so only hb3's short cb chain trails the last MM
                def epi_z(hb):
                    # c = f*ct + i*z into the f tile
                    F, Ii, Z = T(GI_F, hb), T(GI_I, hb), T(GI_Z, hb)
                    nc.vector.tensor_mul(F[:], F[:], cts[hb][:])
                    nc.vector.tensor_mul(Ii[:], Ii[:], Z[:])
                    nc.vector.tensor_add(F[:], F[:], Ii[:])
                    out_dma(c_d, hb, F)
                    if hb == NHB - 1:
                        # tanh(c) into the i tiles; batched to keep the ACT
                        # chain from bubbling on the first DVE c
                        for hb2 in range(NHB):
                            chain(nc.scalar.activation(T(GI_I, hb2)[:],
                                                       T(GI_F, hb2)[:],
                                                       AF.Tanh))

                def epi_o(hb):
                    # h = o * tanh(c) into the tanh(c) (= i) tile
                    Tc = T(GI_I, hb)
                    nc.vector.tensor_mul(Tc[:], Tc[:], T(GI_O, hb)[:])
                    out_dma(h_d, hb, Tc)

                def epi_d(hb):
                    # decay_rate = -ln(sigmoid(-wd)); one Ln table window
                    if hb == NHB - 1:
                        for hb2 in range(NHB):
                            S = T(GI_D, hb2)
                            chain(nc.scalar.activation(S[:], S[:], AF.Ln))
                            nc.vector.tensor_scalar_mul(S[:], S[:], -1.0)
                            out_dma(dr_d, hb2, S)

                def epi_fb(hb, csl=None):
                    # cbar = fb*ct + ib*z into the fb tile
                    FB, IB, Z = T(GI_FB, hb), T(GI_IB, hb), T(GI_Z, hb)
                    if csl is None:
                        csl = slice(0, SLAB)
                    nc.vector.tensor_mul(IB[:, csl], IB[:, csl], Z[:, csl])
                    nc.vector.tensor_mul(FB[:, csl], FB[:, csl],
                                         cts[hb][:, csl])
                    nc.vector.tensor_add(FB[:, csl], FB[:, csl], IB[:, csl])
                    nc.sync.dma_start(
                        cb_d[hb * P:(hb + 1) * P,
                             s * SLAB + csl.start:s * SLAB + csl.stop],
                        FB[:, csl])

                epilogue = {GI_Z: epi_z, GI_O: epi_o,
                            GI_D: epi_d, GI_FB: epi_fb}

                # ---- GEMM + fused bias/activation drain ----
                for grp in GB_GROUPS:
                    accs = {gb: pp.tile([P, SLAB], F32, tag="acc", name="acc")
                            for gb in grp}
                    for k in range(NK):
                        for gb in grp:
                            stat = w_sb[k][:, gb * P:(gb + 1) * P]
                            for h2 in range(SLAB // 512):
                                csl = slice(h2 * 512, (h2 + 1) * 512)
                                nc.tensor.matmul(
                                    accs[gb][:, csl], stat, xh_s[k][:, csl],
                                    start=(k == 0), stop=(k == NK - 1),
                                )
                    for gb in grp:
                        g, hb = gb // NHB, gb % NHB
                        if s == NSLAB - 1 and gb == NGB - 1:
                            # final gate-block: drain + cb chain in halves so
                            # the last output DMA starts as early as possible
                            for h2 in range(SLAB // 512):
                                csl = slice(h2 * 512, (h2 + 1) * 512)
                                chain(nc.scalar.activation(
                                    ga[gb][:, csl], accs[gb][:, csl],
                                    GATE_FUNC[g], bias=bb[:, gb:gb + 1]))
                                epi_fb(hb, csl)
                            continue
                        chain(nc.scalar.activation(
                            ga[gb][:], accs[gb][:], GATE_FUNC[g],
                            scale=-1.0 if g == GI_D else 1.0,
                            bias=bb[:, gb:gb + 1]))
                        if g == GI_O:
                            out_dma(o_d, hb, ga[gb])
                        fn = epilogue.get(g)
                        if fn is not None:
                            fn(hb)

    nc.compile()
    return nc


def kernel(x, ht, ct, Wx, bx, Wh, bh):
    global _nc_cache, LAST_RESULTS
    if _nc_cache is None:
        _nc_cache = _build()
    nc = _nc_cache

    x = np.ascontiguousarray(x, dtype=np.float32)
    ht = np.ascontiguousarray(ht, dtype=np.float32)
    ct = np.ascontiguousarray(ct, dtype=np.float32)

    # host staging: transpose/concat/cast + gate permutation
    xh_full = np.empty((K2, B), dtype=NPBF16)
    xh_full[:I, :] = x.T.astype(NPBF16)
    xh_full[I:, :] = ht.T.astype(NPBF16)
    ctT_full = np.ascontiguousarray(ct.T.astype(NPBF16))

    WxT = np.asarray(Wx, dtype=np.float32).T   # [512, 3584]
    WhT = np.asarray(Wh, dtype=np.float32).T
    bsum = np.asarray(bx, dtype=np.float32) + np.asarray(bh, dtype=np.float32)
    w2 = np.empty((K2, G), dtype=NPBF16)
    bbp = np.empty(G, dtype=np.float32)
    for n, old in enumerate(PERM):
        dsl = slice(n * H, (n + 1) * H)
        ssl = slice(old * H, (old + 1) * H)
        w2[:I, dsl] = WxT[:, ssl].astype(NPBF16)
        w2[I:, dsl] = WhT[:, ssl].astype(NPBF16)
        # d-gate ACT runs with scale=-1: out = sigmoid(-wd) needs -bias
        bbp[dsl] = -bsum[ssl] if n == GI_D else bsum[ssl]
    bias = np.ascontiguousarray(bbp.reshape(NGB, P).T)  # [128, 28]

    in_maps = []
    for cidx in range(NCORES):
        sl = slice(cidx * BS, (cidx + 1) * BS)
        in_maps.append({
            "xh": np.ascontiguousarray(xh_full[:, sl]),
            "w2": w2,
            "ctT": np.ascontiguousarray(ctT_full[:, sl]),
            "bias": bias,
        })

    res = run_bass_kernel_spmd(nc, in_maps, core_ids=list(range(NCORES)),
                               trace=TRACE)
    LAST_RESULTS = res

    outs = {}
    for name in ("h", "c", "cb", "o", "dr"):
        outs[name] = np.concatenate(
            [np.asarray(res.results[cidx][name]).T.astype(np.float32)
             for cidx in range(NCORES)], axis=0
        )
    return outs["h"], outs["c"], outs["cb"], outs["o"], outs["dr"]
